# revision 19
# baseline (speedup 1.0000x reference)
"""Trainium2 Bass kernel for nn_Align_MoE_9732395892816 (moe_routing).

Strategy: 2-way expert-parallel x 4-way token-parallel over 8 NeuronCores.
Core c handles token group (c % 4) [1024 tokens] and expert half (c // 4)
[4 experts]; the host sums the two half-partials per token group. This
halves each core's unique weight stream (67MB vs 134MB in bf16), which
keeps the chip out of the HBM-contention regime where tensor ops stretch
~20% (measured: 134MB/core unique -> 259ns per 512-row matmul; <=67MB ->
216ns).

Per-core device kernel (feature-major activations, tokens on the free axis):
  - expert matmuls in bf16 (fp32 PSUM accumulation); rel-err ~4e-3 total
  - gates computed expert-major [8, T] in full fp32 (top-2 selection must
    match the fp32 reference: min top-2 logit margin on the data is ~2e-5);
    gate weights are expert-PERMUTED per half so the 4 local experts are
    always rows 0-3 (softmax/top-2 are permutation-equivariant); the gate
    chain is interleaved into expert 0's first-layer matmuls
  - per local expert e: hidden = relu(x @ W1[e] + b1[e]), then per output
    d-tile out_j += route[e] * (hidden @ W2[e]) with PE one-hot broadcast of
    the routing row; the local-half bias term sum_e route[e]*b2[e] seeds the
    accumulator via a routeT @ b2 matmul (halves sum to the full bias)
  - outputs are written feature-major [1024, 1024]; the host transposes,
    sums the two expert-half partials, and reassembles

kernel(**inputs) marshals the full inputs, runs the SPMD NEFF on cores 0-7,
and reassembles the full (out0, out1) tuple exactly like the reference.
"""

import os
import sys

for _p in ("/opt/trn_rl_repo",):
    if _p not in sys.path:
        sys.path.insert(0, _p)

import ml_dtypes
import numpy as np

import concourse.mybir as mybir
import concourse.tile as tile
from concourse import bacc
from concourse.bass import ts
from concourse.bass_utils import run_bass_kernel_spmd
from concourse import bass_isa

F32 = mybir.dt.float32
BF16 = mybir.dt.bfloat16
P = 128

# problem sizes (hardcoded per spec)
B, S, D, E, TOPK = 8, 512, 2048, 8, 2
NCORES = 8
EP = 2                        # expert-parallel degree
DP = NCORES // EP             # token-parallel degree
T = B * S // DP               # tokens per core (1024)
EL = E // EP                  # local experts per core (4)
H = D // 2

LAST_EXEC_TIME_NS = None     # set when MOE_TRACE=1


def _build_moe(T, D, w_bufs=4, psum_bufs=3, psumb_bufs=4, acc_bufs=3):
    """Build + bacc-compile the per-core module."""
    KT = D // P            # k-tiles over model dim (also h-tile count)
    H = D // 2
    HKo = H // P           # gate contraction k-tiles
    JT = KT                # output d-tiles (both halves)
    JH = JT // 2
    TC = T // 512          # 512-token chunks (psum-bank width)
    AF = mybir.ActivationFunctionType

    nc = bacc.Bacc()
    MMD = BF16
    xb = nc.dram_tensor("xb", [D, T], MMD, kind="ExternalInput")
    xf = nc.dram_tensor("xf", [D, T], F32, kind="ExternalInput")
    w1r = nc.dram_tensor("w1r", [EL, KT, P, KT, P], MMD, kind="ExternalInput")
    w2r = nc.dram_tensor("w2r", [EL, JT, P, KT, P], MMD, kind="ExternalInput")
    wg = nc.dram_tensor("wg", [P, HKo, E], F32, kind="ExternalInput")
    wf = nc.dram_tensor("wf", [P, HKo, E], F32, kind="ExternalInput")
    bgt = nc.dram_tensor("bgt", [1, E], F32, kind="ExternalInput")
    bft = nc.dram_tensor("bft", [1, E], F32, kind="ExternalInput")
    b1r = nc.dram_tensor("b1r", [P, EL, KT], F32, kind="ExternalInput")
    b2t = nc.dram_tensor("b2t", [EL, D], MMD, kind="ExternalInput")
    wv = nc.dram_tensor("wv", [1, 2], F32, kind="ExternalInput")
    selt = nc.dram_tensor("selt", [E, EL, P], MMD, kind="ExternalInput")
    y0 = nc.dram_tensor("y0", [H, T], BF16, kind="ExternalOutput")
    y1 = nc.dram_tensor("y1", [H, T], BF16, kind="ExternalOutput")

    with tile.TileContext(nc) as tc:
        with (
            tc.tile_pool(name="const", bufs=1) as cpool,
            tc.tile_pool(name="wpool1", bufs=w_bufs - 1) as w1pool,
            tc.tile_pool(name="wpool2", bufs=w_bufs) as w2pool,
            tc.tile_pool(name="xfp", bufs=3) as xfpool,
            tc.tile_pool(name="bcastp", bufs=1) as bpool,
            tc.tile_pool(name="accp", bufs=acc_bufs) as apool,
            tc.tile_pool(name="psA", bufs=psumb_bufs, space="PSUM") as psumA,
            tc.tile_pool(name="psB", bufs=psum_bufs, space="PSUM") as psumB,
            tc.tile_pool(name="psC", bufs=1, space="PSUM") as psumC,
            tc.tile_pool(name="gsb", bufs=1) as gsb,
        ):
            # ---- persistent tiles ----
            # small tensors first so they don't queue behind the big X DMAs
            wg_sb = cpool.tile([P, HKo, E], F32)
            nc.sync.dma_start(wg_sb[:], wg[:])
            wf_sb = cpool.tile([P, HKo, E], F32)
            nc.sync.dma_start(wf_sb[:], wf[:])
            bg8 = cpool.tile([E, 1], F32)
            nc.sync.dma_start(bg8[:], bgt.rearrange("o e -> e o"))
            bf8 = cpool.tile([E, 1], F32)
            nc.sync.dma_start(bf8[:], bft.rearrange("o e -> e o"))
            b1_sb = cpool.tile([P, EL, KT], F32)
            nc.sync.dma_start(b1_sb[:], b1r[:])
            b2_sb = cpool.tile([EL, D], MMD)
            nc.sync.dma_start(b2_sb[:], b2t[:])
            wv_sb = cpool.tile([1, 2], F32)
            nc.sync.dma_start(wv_sb[:], wv[:])
            sel = cpool.tile([E, EL, P], MMD)
            nc.sync.dma_start(sel[:], selt[:])
            ones_sb = cpool.tile([1, P], F32)
            nc.vector.memset(ones_sb, 1.0)
            ones8 = cpool.tile([E, E], F32)
            nc.vector.memset(ones8, 1.0)
            routeTg = cpool.tile([E, T], MMD)
            routeTf = cpool.tile([E, T], MMD)
            out_sb = cpool.tile([P, JT, T], MMD)
            htmp_a = cpool.tile([P, KT, T], MMD)
            htmp_b = cpool.tile([P, KT, T], MMD)
            htmp2 = [htmp_a, htmp_b]

            # prefetch the first expert's first W1 tiles ahead of the X DMAs
            # so the first A chain isn't stuck behind 4MB of queue
            prefetched = {}
            for hk in (0, 1):
                w1t = w1pool.tile([P, KT, P], MMD, tag="w1t")
                nc.sync.dma_start(w1t[:], w1r[0, hk])
                prefetched[(0, hk)] = w1t

            XT = cpool.tile([P, KT, T], MMD)
            xb_r = xb.rearrange("(ko p) t -> p ko t", p=P)
            # token-chunk 0 of every k-tile first: the first A chains need it
            for tch in range(TC):
                for ko in range(KT):
                    nc.sync.dma_start(XT[:, ko, ts(tch, 512)],
                                      xb_r[:, ko, ts(tch, 512)])
            xf_r = xf.rearrange("(ko p) t -> p ko t", p=P)

            def phase_a(e, hk_list, hb, gen=None):
                # hidden = relu(x @ W1[e] + b1[e]) -> htmp (feature-major)
                htmp = htmp2[hb]
                for hk in hk_list:
                    w1t = prefetched.pop((e, hk), None)
                    if w1t is None:
                        w1t = w1pool.tile([P, KT, P], MMD, tag="w1t")
                        nc.sync.dma_start(w1t[:], w1r[e, hk])
                    for tch in range(TC):
                        psh = psumA.tile([P, 512], F32, tag="psh")
                        for dk in range(KT):
                            nc.tensor.matmul(
                                psh,
                                lhsT=w1t[:, dk, :],
                                rhs=XT[:, dk, ts(tch, 512)],
                                start=(dk == 0),
                                stop=(dk == KT - 1),
                            )
                        nc.scalar.activation(htmp[:, hk, ts(tch, 512)], psh,
                                             AF.Relu, bias=b1_sb[:, e, hk:hk + 1])
                        if gen is not None:
                            # one unit of gate work per A chain: keeps the
                            # PE fed while the gate stream/top-2 chain runs
                            next(gen, None)

            # `weight` scalars broadcast across partitions (tiny, PE)
            wvb_ps = psumA.tile([P, 2], F32, tag="psh")
            nc.tensor.matmul(wvb_ps, ones_sb, wv_sb, start=True, stop=True)
            wvb = cpool.tile([P, 2], F32)
            nc.vector.tensor_copy(wvb, wvb_ps)

            RO = bass_isa.ReduceOp

            def gate_gen():
                """Gate logits (full fp32, streamed XTF tiles), softmax, and
                top-2 sparsify, emitted one small unit per yield so phase_a
                can interleave it between its chains (the XTF stream then has
                ~7us per tile instead of underrunning, and the slow 8-lane
                vector/gpsimd chain hides in the PE shadow)."""
                exv = {}
                for which, w_sb, bias8, ko0 in (("g", wg_sb, bg8, 0),
                                                ("f", wf_sb, bf8, HKo)):
                    ex = gsb.tile([E, T], F32, tag=f"gb{which}", name=f"ex{which}")
                    for tch in range(TC):
                        psg = psumC.tile([E, 512], F32, tag="bps", name="psg")
                        tiles = []
                        for ko in range(2):
                            xft = xfpool.tile([P, 512], F32, tag="xf", name="xft")
                            nc.sync.dma_start(xft[:],
                                              xf_r[:, ko0 + ko, ts(tch, 512)])
                            tiles.append(xft)
                        for ko in range(HKo):
                            if ko + 2 < HKo:
                                xft = xfpool.tile([P, 512], F32, tag="xf",
                                                  name="xft")
                                nc.sync.dma_start(
                                    xft[:], xf_r[:, ko0 + ko + 2, ts(tch, 512)])
                                tiles.append(xft)
                            nc.tensor.matmul(psg,
                                             lhsT=w_sb[:, ko, :],
                                             rhs=tiles[ko][:],
                                             start=(ko == 0),
                                             stop=(ko == HKo - 1))
                            if ko % 2 == 1:
                                yield
                        # exp(logit + bias); logits are O(1), no max-subtract
                        nc.scalar.activation(ex[:, ts(tch, 512)], psg, AF.Exp,
                                             bias=bias8[:, 0:1])
                        yield
                    exv[which] = ex

                # softmax denominator via a tiny all-ones matmul, then
                # normalize in place
                rout = {}
                for which in ("g", "f"):
                    rcp = gsb.tile([E, T], F32, tag="grc", name="rcp")
                    for tch in range(TC):
                        pss = psumC.tile([E, 512], F32, tag="bps", name="pss")
                        nc.tensor.matmul(pss, lhsT=ones8[:, :],
                                         rhs=exv[which][:, ts(tch, 512)],
                                         start=True, stop=True)
                        nc.vector.reciprocal(rcp[:, ts(tch, 512)], pss)
                        yield
                    nc.vector.tensor_mul(exv[which], exv[which], rcp)
                    rout[which] = exv[which]
                    yield

                # top-2 sparsify g: zero entries below the 2nd-largest prob
                mx1 = gsb.tile([E, T], F32, tag="red", name="mx1")
                nc.gpsimd.partition_all_reduce(mx1[:], rout["g"][:], channels=E,
                                               reduce_op=RO.max)
                yield
                msk = gsb.tile([E, T], F32, tag="gb2", name="msk")
                nc.vector.tensor_tensor(msk, rout["g"], mx1,
                                        mybir.AluOpType.is_ge)
                yield
                nc.vector.tensor_scalar_mul(msk, msk, 1e30)
                yield
                nc.vector.tensor_sub(msk, rout["g"], msk)
                yield
                mx2 = gsb.tile([E, T], F32, tag="red", name="mx2")
                nc.gpsimd.partition_all_reduce(mx2[:], msk[:], channels=E,
                                               reduce_op=RO.max)
                yield
                keep = gsb.tile([E, T], F32, tag="gb2", name="keep")
                nc.vector.tensor_tensor(keep, rout["g"], mx2,
                                        mybir.AluOpType.is_ge)
                yield
                nc.vector.tensor_scalar_mul(keep, keep, wvb[0:E, 0:1])
                yield
                nc.vector.tensor_mul(routeTg[:, :], rout["g"], keep)
                nc.vector.tensor_scalar_mul(routeTf[:, :], rout["f"],
                                            wvb[0:E, 1:2])

            # ---- experts 0+1 layer 1, gate chain interleaved ----
            gen = gate_gen()
            phase_a(0, range(KT), 0, gen)
            phase_a(1, range(KT), 1, gen)
            for _ in gen:   # drain any remaining gate work
                pass

            # ---- bias init: out_sb[j] = routeT[local] @ b2 chunk ----
            # (only the 4 local experts' bias terms; the two halves sum to
            # the full bias on the host)
            for j in range(JT):
                routeT = routeTg if j < JH else routeTf
                for tch in range(TC):
                    psb = psumB.tile([P, 512], F32, tag="pso")
                    nc.tensor.matmul(psb, lhsT=b2_sb[:, ts(j, P)],
                                     rhs=routeT[0:EL, ts(tch, 512)],
                                     start=True, stop=True)
                    nc.vector.tensor_copy(out_sb[:, j, ts(tch, 512)], psb)

            # ---- expert loop (local experts are rows 0..EL-1) ----
            for e in range(EL):
                # broadcast routing rows across partitions via PE one-hot
                bgb = bpool.tile([P, T], F32, tag="bgb")
                bfb = bpool.tile([P, T], F32, tag="bfb")
                for bdst, rT in ((bgb, routeTg), (bfb, routeTf)):
                    for tch in range(TC):
                        bps = psumC.tile([P, 512], F32, tag="bps")
                        nc.tensor.matmul(bps, lhsT=sel[:, e, :],
                                         rhs=rT[:, ts(tch, 512)],
                                         start=True, stop=True)
                        nc.vector.tensor_copy(bdst[:, ts(tch, 512)], bps)

                hb = e % 2
                if e >= 2:
                    phase_a(e, range(KT), hb)
                htmp = htmp2[hb]

                # Phase B: out_j += route[e] * (hidden @ W2[e] chunk)
                for j in range(JT):
                    w2t = w2pool.tile([P, KT, P], MMD, tag="w2t")
                    nc.sync.dma_start(w2t[:], w2r[e, j])
                    bsrc = bgb if j < JH else bfb
                    for tch in range(TC):
                        pso = psumB.tile([P, 512], F32, tag="pso")
                        for hk in range(KT):
                            nc.tensor.matmul(
                                pso,
                                lhsT=w2t[:, hk, :],
                                rhs=htmp[:, hk, ts(tch, 512)],
                                start=(hk == 0),
                                stop=(hk == KT - 1),
                            )
                        tmp = apool.tile([P, 512], MMD, tag="acc")
                        nc.vector.tensor_mul(tmp, pso, bsrc[:, ts(tch, 512)])
                        nc.vector.tensor_add(out_sb[:, j, ts(tch, 512)],
                                             out_sb[:, j, ts(tch, 512)], tmp)
                        if e == EL - 1:
                            # final value for this d-tile chunk: stream it out
                            if j < JH:
                                nc.sync.dma_start(
                                    y0[ts(j, P), ts(tch, 512)],
                                    out_sb[:, j, ts(tch, 512)])
                            else:
                                nc.sync.dma_start(
                                    y1[ts(j - JH, P), ts(tch, 512)],
                                    out_sb[:, j, ts(tch, 512)])

    nc.compile()
    return nc


_NC_CACHE = {}


def _get_nc():
    if "nc" not in _NC_CACHE:
        _NC_CACHE["nc"] = _build_moe(T, D)
    return _NC_CACHE["nc"]


def _fingerprint(*arrays):
    parts = []
    for a in arrays:
        a = np.asarray(a)
        flat = a.reshape(-1)
        step = max(1, flat.size // 64)
        parts.append((id(a), a.shape, flat[::step][:64].tobytes()))
    return hash(tuple((i, s, b) for i, s, b in parts))


def _prep_shared(Wg, bg, Wf, bf, W1, b1, W2, b2, weight):
    """Per-expert-half input dicts. Gate tensors are expert-permuted so the
    half's 4 local experts are rows 0-3."""
    KT = D // P
    HKo = H // P
    f32 = np.float32
    bf16 = ml_dtypes.bfloat16
    halves = []
    for h in range(EP):
        loc = list(range(h * EL, (h + 1) * EL))
        rem = [e for e in range(E) if e not in loc]
        perm = loc + rem
        sel_np = np.zeros((E, EL, P), f32)
        for i in range(EL):
            sel_np[i, i, :] = 1.0
        halves.append({
            "w1r": np.ascontiguousarray(
                W1[loc].reshape(EL, KT, P, KT, P).transpose(0, 3, 2, 1, 4)
            ).astype(bf16),
            "w2r": np.ascontiguousarray(
                W2[loc].reshape(EL, KT, P, KT, P).transpose(0, 3, 2, 1, 4)
            ).astype(bf16),
            "wg": np.ascontiguousarray(
                Wg[:, perm].reshape(HKo, P, E).transpose(1, 0, 2)).astype(f32, copy=False),
            "wf": np.ascontiguousarray(
                Wf[:, perm].reshape(HKo, P, E).transpose(1, 0, 2)).astype(f32, copy=False),
            "bgt": np.ascontiguousarray(np.asarray(bg, f32)[perm].reshape(1, E)),
            "bft": np.ascontiguousarray(np.asarray(bf, f32)[perm].reshape(1, E)),
            "b1r": np.ascontiguousarray(
                b1[loc].reshape(EL, KT, P).transpose(2, 0, 1)).astype(f32, copy=False),
            "b2t": np.asarray(b2, f32)[loc].astype(bf16),
            "wv": np.ascontiguousarray(np.asarray(weight, f32).reshape(1, 2)),
            "selt": sel_np.astype(bf16),
        })
    return halves


def kernel(vector, Wg, bg, Wf, bf, W1, b1, W2, b2, weight, top_k):
    """Full inputs in, full output out (tuple (out0, out1), matching the
    reference)."""
    global LAST_EXEC_TIME_NS
    assert int(top_k) == TOPK, f"kernel compiled for top_k={TOPK}"
    vector = np.asarray(vector, np.float32)
    assert vector.shape == (B, S, D), vector.shape

    nc = _get_nc()
    fp = _fingerprint(Wg, bg, Wf, bf, W1, b1, W2, b2, weight)
    if _NC_CACHE.get("shared_fp") != fp:
        _NC_CACHE["shared"] = _prep_shared(
            np.asarray(Wg, np.float32), bg, np.asarray(Wf, np.float32), bf,
            np.asarray(W1, np.float32), np.asarray(b1, np.float32),
            np.asarray(W2, np.float32), np.asarray(b2, np.float32), weight)
        _NC_CACHE["shared_fp"] = fp
    halves = _NC_CACHE["shared"]

    tokens = vector.reshape(B * S, D)
    xts = []
    for g in range(DP):
        xt = np.ascontiguousarray(tokens[g * T:(g + 1) * T].T)
        xts.append((xt, xt.astype(ml_dtypes.bfloat16)))
    in_maps = []
    for c in range(NCORES):
        h, g = divmod(c, DP)
        m = dict(halves[h])
        m["xf"], m["xb"] = xts[g]
        in_maps.append(m)

    trace = bool(os.environ.get("MOE_TRACE"))
    res = run_bass_kernel_spmd(nc, in_maps, core_ids=list(range(NCORES)),
                               trace=trace)
    if trace:
        LAST_EXEC_TIME_NS = res.exec_time_ns

    out0 = np.empty((B * S, H), np.float32)
    out1 = np.empty((B * S, H), np.float32)
    for g in range(DP):
        sl = slice(g * T, (g + 1) * T)
        out0[sl] = (res.results[g]["y0"].T.astype(np.float32)
                    + res.results[DP + g]["y0"].T.astype(np.float32))
        out1[sl] = (res.results[g]["y1"].T.astype(np.float32)
                    + res.results[DP + g]["y1"].T.astype(np.float32))
    return (np.ascontiguousarray(out0.reshape(B, S, H)),
            np.ascontiguousarray(out1.reshape(B, S, H)))


# revision 20
# speedup vs baseline: 1.0197x; 1.0197x over previous
"""Trainium2 Bass kernel for nn_Align_MoE_9732395892816 (moe_routing).

Strategy: 2-way expert-parallel x 4-way token-parallel over 8 NeuronCores.
Core c handles token group (c % 4) [1024 tokens] and expert half (c // 4)
[4 experts]; the host sums the two half-partials per token group. This
halves each core's unique weight stream (67MB vs 134MB in bf16), which
keeps the chip out of the HBM-contention regime where tensor ops stretch
~20% (measured: 134MB/core unique -> 259ns per 512-row matmul; <=67MB ->
216ns).

Per-core device kernel (feature-major activations, tokens on the free axis):
  - expert matmuls in bf16 (fp32 PSUM accumulation); rel-err ~4e-3 total
  - gates computed expert-major [8, T] in full fp32 (top-2 selection must
    match the fp32 reference: min top-2 logit margin on the data is ~2e-5);
    gate weights are expert-PERMUTED per half so the 4 local experts are
    always rows 0-3 (softmax/top-2 are permutation-equivariant); the gate
    chain is interleaved into expert 0's first-layer matmuls
  - per local expert e: hidden = relu(x @ W1[e] + b1[e]), then per output
    d-tile out_j += route[e] * (hidden @ W2[e]) with PE one-hot broadcast of
    the routing row; the local-half bias term sum_e route[e]*b2[e] seeds the
    accumulator via a routeT @ b2 matmul (halves sum to the full bias)
  - outputs are written feature-major [1024, 1024]; the host transposes,
    sums the two expert-half partials, and reassembles

kernel(**inputs) marshals the full inputs, runs the SPMD NEFF on cores 0-7,
and reassembles the full (out0, out1) tuple exactly like the reference.
"""

import os
import sys

for _p in ("/opt/trn_rl_repo",):
    if _p not in sys.path:
        sys.path.insert(0, _p)

import ml_dtypes
import numpy as np

import concourse.mybir as mybir
import concourse.tile as tile
from concourse import bacc
from concourse.bass import ts
from concourse.bass_utils import run_bass_kernel_spmd
from concourse import bass_isa

F32 = mybir.dt.float32
BF16 = mybir.dt.bfloat16
P = 128

# problem sizes (hardcoded per spec)
B, S, D, E, TOPK = 8, 512, 2048, 8, 2
NCORES = 8
EP = 2                        # expert-parallel degree
DP = NCORES // EP             # token-parallel degree
T = B * S // DP               # tokens per core (1024)
EL = E // EP                  # local experts per core (4)
H = D // 2

LAST_EXEC_TIME_NS = None     # set when MOE_TRACE=1


def _build_moe(T, D, w_bufs=4, psum_bufs=3, psumb_bufs=3, acc_bufs=3):
    """Build + bacc-compile the per-core module."""
    KT = D // P            # k-tiles over model dim (also h-tile count)
    H = D // 2
    HKo = H // P           # gate contraction k-tiles
    JT = KT                # output d-tiles (both halves)
    JH = JT // 2
    TC = T // 512          # 512-token chunks (psum-bank width)
    AF = mybir.ActivationFunctionType

    nc = bacc.Bacc()
    MMD = BF16
    xb = nc.dram_tensor("xb", [D, T], MMD, kind="ExternalInput")
    xf = nc.dram_tensor("xf", [D, T], F32, kind="ExternalInput")
    w1r = nc.dram_tensor("w1r", [EL, KT, P, KT, P], MMD, kind="ExternalInput")
    w2r = nc.dram_tensor("w2r", [EL, JT, P, KT, P], MMD, kind="ExternalInput")
    wg = nc.dram_tensor("wg", [P, HKo, E], F32, kind="ExternalInput")
    wf = nc.dram_tensor("wf", [P, HKo, E], F32, kind="ExternalInput")
    bgt = nc.dram_tensor("bgt", [1, E], F32, kind="ExternalInput")
    bft = nc.dram_tensor("bft", [1, E], F32, kind="ExternalInput")
    b1r = nc.dram_tensor("b1r", [P, EL, KT], F32, kind="ExternalInput")
    b2t = nc.dram_tensor("b2t", [EL, D], MMD, kind="ExternalInput")
    wv = nc.dram_tensor("wv", [1, 2], F32, kind="ExternalInput")
    selt = nc.dram_tensor("selt", [E, EL, P], MMD, kind="ExternalInput")
    y0 = nc.dram_tensor("y0", [H, T], BF16, kind="ExternalOutput")
    y1 = nc.dram_tensor("y1", [H, T], BF16, kind="ExternalOutput")

    with tile.TileContext(nc) as tc:
        with (
            tc.tile_pool(name="const", bufs=1) as cpool,
            tc.tile_pool(name="wpool1", bufs=w_bufs - 1) as w1pool,
            tc.tile_pool(name="wpool2", bufs=w_bufs) as w2pool,
            tc.tile_pool(name="xfp", bufs=3) as xfpool,
            tc.tile_pool(name="bcastp", bufs=1) as bpool,
            tc.tile_pool(name="accp", bufs=acc_bufs) as apool,
            tc.tile_pool(name="psA", bufs=psumb_bufs, space="PSUM") as psumA,
            tc.tile_pool(name="psB", bufs=psum_bufs, space="PSUM") as psumB,
            tc.tile_pool(name="psC", bufs=2, space="PSUM") as psumC,
            tc.tile_pool(name="gsb", bufs=1) as gsb,
        ):
            # ---- persistent tiles ----
            # small tensors first so they don't queue behind the big X DMAs
            wg_sb = cpool.tile([P, HKo, E], F32)
            nc.sync.dma_start(wg_sb[:], wg[:])
            wf_sb = cpool.tile([P, HKo, E], F32)
            nc.sync.dma_start(wf_sb[:], wf[:])
            bg8 = cpool.tile([E, 1], F32)
            nc.sync.dma_start(bg8[:], bgt.rearrange("o e -> e o"))
            bf8 = cpool.tile([E, 1], F32)
            nc.sync.dma_start(bf8[:], bft.rearrange("o e -> e o"))
            b1_sb = cpool.tile([P, EL, KT], F32)
            nc.sync.dma_start(b1_sb[:], b1r[:])
            b2_sb = cpool.tile([EL, D], MMD)
            nc.sync.dma_start(b2_sb[:], b2t[:])
            wv_sb = cpool.tile([1, 2], F32)
            nc.sync.dma_start(wv_sb[:], wv[:])
            sel = cpool.tile([E, EL, P], MMD)
            nc.sync.dma_start(sel[:], selt[:])
            ones_sb = cpool.tile([1, P], F32)
            nc.vector.memset(ones_sb, 1.0)
            ones8 = cpool.tile([E, E], F32)
            nc.vector.memset(ones8, 1.0)
            routeTg = cpool.tile([E, T], MMD)
            routeTf = cpool.tile([E, T], MMD)
            out_sb = cpool.tile([P, JT, T], MMD)
            htmp_a = cpool.tile([P, KT, T], MMD)
            htmp_b = cpool.tile([P, KT, T], MMD)
            htmp2 = [htmp_a, htmp_b]

            # prefetch the first expert's first W1 tiles ahead of the X DMAs
            # so the first A chain isn't stuck behind 4MB of queue
            prefetched = {}
            for hk in (0, 1):
                w1t = w1pool.tile([P, KT, P], MMD, tag="w1t")
                nc.sync.dma_start(w1t[:], w1r[0, hk])
                prefetched[(0, hk)] = w1t

            XT = cpool.tile([P, KT, T], MMD)
            xb_r = xb.rearrange("(ko p) t -> p ko t", p=P)
            # token-chunk 0 of every k-tile first: the first A chains need it
            for tch in range(TC):
                for ko in range(KT):
                    nc.sync.dma_start(XT[:, ko, ts(tch, 512)],
                                      xb_r[:, ko, ts(tch, 512)])
            xf_r = xf.rearrange("(ko p) t -> p ko t", p=P)

            def phase_a(e, hk_list, hb, gen=None):
                # hidden = relu(x @ W1[e] + b1[e]) -> htmp (feature-major)
                htmp = htmp2[hb]
                for hk in hk_list:
                    w1t = prefetched.pop((e, hk), None)
                    if w1t is None:
                        w1t = w1pool.tile([P, KT, P], MMD, tag="w1t")
                        nc.sync.dma_start(w1t[:], w1r[e, hk])
                    for tch in range(TC):
                        psh = psumA.tile([P, 512], F32, tag="psh")
                        for dk in range(KT):
                            nc.tensor.matmul(
                                psh,
                                lhsT=w1t[:, dk, :],
                                rhs=XT[:, dk, ts(tch, 512)],
                                start=(dk == 0),
                                stop=(dk == KT - 1),
                            )
                        nc.scalar.activation(htmp[:, hk, ts(tch, 512)], psh,
                                             AF.Relu, bias=b1_sb[:, e, hk:hk + 1])
                        if gen is not None:
                            # one unit of gate work per A chain: keeps the
                            # PE fed while the gate stream/top-2 chain runs
                            next(gen, None)

            # `weight` scalars broadcast across partitions (tiny, PE)
            wvb_ps = psumA.tile([P, 2], F32, tag="psh")
            nc.tensor.matmul(wvb_ps, ones_sb, wv_sb, start=True, stop=True)
            wvb = cpool.tile([P, 2], F32)
            nc.vector.tensor_copy(wvb, wvb_ps)

            RO = bass_isa.ReduceOp

            def gate_gen():
                """Gate logits (full fp32, streamed XTF tiles), softmax, and
                top-2 sparsify, emitted one small unit per yield so phase_a
                can interleave it between its chains (the XTF stream then has
                ~7us per tile instead of underrunning, and the slow 8-lane
                vector/gpsimd chain hides in the PE shadow)."""
                exv = {}
                for which, w_sb, bias8, ko0 in (("g", wg_sb, bg8, 0),
                                                ("f", wf_sb, bf8, HKo)):
                    ex = gsb.tile([E, T], F32, tag=f"gb{which}", name=f"ex{which}")
                    for tch in range(TC):
                        psg = psumC.tile([E, 512], F32, tag="bps", name="psg")
                        tiles = []
                        for ko in range(2):
                            xft = xfpool.tile([P, 512], F32, tag="xf", name="xft")
                            nc.sync.dma_start(xft[:],
                                              xf_r[:, ko0 + ko, ts(tch, 512)])
                            tiles.append(xft)
                        for ko in range(HKo):
                            if ko + 2 < HKo:
                                xft = xfpool.tile([P, 512], F32, tag="xf",
                                                  name="xft")
                                nc.sync.dma_start(
                                    xft[:], xf_r[:, ko0 + ko + 2, ts(tch, 512)])
                                tiles.append(xft)
                            nc.tensor.matmul(psg,
                                             lhsT=w_sb[:, ko, :],
                                             rhs=tiles[ko][:],
                                             start=(ko == 0),
                                             stop=(ko == HKo - 1))
                            if ko % 2 == 1:
                                yield
                        # exp(logit + bias); logits are O(1), no max-subtract
                        nc.scalar.activation(ex[:, ts(tch, 512)], psg, AF.Exp,
                                             bias=bias8[:, 0:1])
                        yield
                    exv[which] = ex

                # softmax denominator via a tiny all-ones matmul, then
                # normalize in place
                rout = {}
                for which in ("g", "f"):
                    rcp = gsb.tile([E, T], F32, tag="grc", name="rcp")
                    for tch in range(TC):
                        pss = psumC.tile([E, 512], F32, tag="bps", name="pss")
                        nc.tensor.matmul(pss, lhsT=ones8[:, :],
                                         rhs=exv[which][:, ts(tch, 512)],
                                         start=True, stop=True)
                        nc.vector.reciprocal(rcp[:, ts(tch, 512)], pss)
                        yield
                    nc.vector.tensor_mul(exv[which], exv[which], rcp)
                    rout[which] = exv[which]
                    yield

                # top-2 sparsify g: zero entries below the 2nd-largest prob
                mx1 = gsb.tile([E, T], F32, tag="red", name="mx1")
                nc.gpsimd.partition_all_reduce(mx1[:], rout["g"][:], channels=E,
                                               reduce_op=RO.max)
                yield
                msk = gsb.tile([E, T], F32, tag="gb2", name="msk")
                nc.vector.tensor_tensor(msk, rout["g"], mx1,
                                        mybir.AluOpType.is_ge)
                yield
                nc.vector.tensor_scalar_mul(msk, msk, 1e30)
                yield
                nc.vector.tensor_sub(msk, rout["g"], msk)
                yield
                mx2 = gsb.tile([E, T], F32, tag="red", name="mx2")
                nc.gpsimd.partition_all_reduce(mx2[:], msk[:], channels=E,
                                               reduce_op=RO.max)
                yield
                keep = gsb.tile([E, T], F32, tag="gb2", name="keep")
                nc.vector.tensor_tensor(keep, rout["g"], mx2,
                                        mybir.AluOpType.is_ge)
                yield
                nc.vector.tensor_scalar_mul(keep, keep, wvb[0:E, 0:1])
                yield
                nc.vector.tensor_mul(routeTg[:, :], rout["g"], keep)
                nc.vector.tensor_scalar_mul(routeTf[:, :], rout["f"],
                                            wvb[0:E, 1:2])

            # ---- experts 0+1 layer 1, gate chain interleaved ----
            gen = gate_gen()
            phase_a(0, range(KT), 0, gen)
            phase_a(1, range(KT), 1, gen)
            for _ in gen:   # drain any remaining gate work
                pass

            # ---- bias init: out_sb[j] = routeT[local] @ b2 chunk ----
            # (only the 4 local experts' bias terms; the two halves sum to
            # the full bias on the host)
            for j in range(JT):
                routeT = routeTg if j < JH else routeTf
                for tch in range(TC):
                    psb = psumB.tile([P, 512], F32, tag="pso")
                    nc.tensor.matmul(psb, lhsT=b2_sb[:, ts(j, P)],
                                     rhs=routeT[0:EL, ts(tch, 512)],
                                     start=True, stop=True)
                    nc.vector.tensor_copy(out_sb[:, j, ts(tch, 512)], psb)

            # ---- expert loop (local experts are rows 0..EL-1) ----
            for e in range(EL):
                # broadcast routing rows across partitions via PE one-hot
                bgb = bpool.tile([P, T], F32, tag="bgb")
                bfb = bpool.tile([P, T], F32, tag="bfb")
                for bdst, rT in ((bgb, routeTg), (bfb, routeTf)):
                    for tch in range(TC):
                        bps = psumC.tile([P, 512], F32, tag="bps")
                        nc.tensor.matmul(bps, lhsT=sel[:, e, :],
                                         rhs=rT[:, ts(tch, 512)],
                                         start=True, stop=True)
                        nc.vector.tensor_copy(bdst[:, ts(tch, 512)], bps)

                hb = e % 2
                if e >= 2:
                    phase_a(e, range(KT), hb)
                htmp = htmp2[hb]

                # Phase B: out_j += route[e] * (hidden @ W2[e] chunk)
                for j in range(JT):
                    w2t = w2pool.tile([P, KT, P], MMD, tag="w2t")
                    nc.sync.dma_start(w2t[:], w2r[e, j])
                    bsrc = bgb if j < JH else bfb
                    for tch in range(TC):
                        pso = psumB.tile([P, 512], F32, tag="pso")
                        for hk in range(KT):
                            nc.tensor.matmul(
                                pso,
                                lhsT=w2t[:, hk, :],
                                rhs=htmp[:, hk, ts(tch, 512)],
                                start=(hk == 0),
                                stop=(hk == KT - 1),
                            )
                        tmp = apool.tile([P, 512], MMD, tag="acc")
                        nc.vector.tensor_mul(tmp, pso, bsrc[:, ts(tch, 512)])
                        nc.vector.tensor_add(out_sb[:, j, ts(tch, 512)],
                                             out_sb[:, j, ts(tch, 512)], tmp)
                        if e == EL - 1:
                            # final value for this d-tile chunk: stream it out
                            if j < JH:
                                nc.sync.dma_start(
                                    y0[ts(j, P), ts(tch, 512)],
                                    out_sb[:, j, ts(tch, 512)])
                            else:
                                nc.sync.dma_start(
                                    y1[ts(j - JH, P), ts(tch, 512)],
                                    out_sb[:, j, ts(tch, 512)])

    nc.compile()
    return nc


_NC_CACHE = {}


def _get_nc():
    if "nc" not in _NC_CACHE:
        _NC_CACHE["nc"] = _build_moe(T, D)
    return _NC_CACHE["nc"]


def _fingerprint(*arrays):
    parts = []
    for a in arrays:
        a = np.asarray(a)
        flat = a.reshape(-1)
        step = max(1, flat.size // 64)
        parts.append((id(a), a.shape, flat[::step][:64].tobytes()))
    return hash(tuple((i, s, b) for i, s, b in parts))


def _prep_shared(Wg, bg, Wf, bf, W1, b1, W2, b2, weight):
    """Per-expert-half input dicts. Gate tensors are expert-permuted so the
    half's 4 local experts are rows 0-3."""
    KT = D // P
    HKo = H // P
    f32 = np.float32
    bf16 = ml_dtypes.bfloat16
    halves = []
    for h in range(EP):
        loc = list(range(h * EL, (h + 1) * EL))
        rem = [e for e in range(E) if e not in loc]
        perm = loc + rem
        sel_np = np.zeros((E, EL, P), f32)
        for i in range(EL):
            sel_np[i, i, :] = 1.0
        halves.append({
            "w1r": np.ascontiguousarray(
                W1[loc].reshape(EL, KT, P, KT, P).transpose(0, 3, 2, 1, 4)
            ).astype(bf16),
            "w2r": np.ascontiguousarray(
                W2[loc].reshape(EL, KT, P, KT, P).transpose(0, 3, 2, 1, 4)
            ).astype(bf16),
            "wg": np.ascontiguousarray(
                Wg[:, perm].reshape(HKo, P, E).transpose(1, 0, 2)).astype(f32, copy=False),
            "wf": np.ascontiguousarray(
                Wf[:, perm].reshape(HKo, P, E).transpose(1, 0, 2)).astype(f32, copy=False),
            "bgt": np.ascontiguousarray(np.asarray(bg, f32)[perm].reshape(1, E)),
            "bft": np.ascontiguousarray(np.asarray(bf, f32)[perm].reshape(1, E)),
            "b1r": np.ascontiguousarray(
                b1[loc].reshape(EL, KT, P).transpose(2, 0, 1)).astype(f32, copy=False),
            "b2t": np.asarray(b2, f32)[loc].astype(bf16),
            "wv": np.ascontiguousarray(np.asarray(weight, f32).reshape(1, 2)),
            "selt": sel_np.astype(bf16),
        })
    return halves


def kernel(vector, Wg, bg, Wf, bf, W1, b1, W2, b2, weight, top_k):
    """Full inputs in, full output out (tuple (out0, out1), matching the
    reference)."""
    global LAST_EXEC_TIME_NS
    assert int(top_k) == TOPK, f"kernel compiled for top_k={TOPK}"
    vector = np.asarray(vector, np.float32)
    assert vector.shape == (B, S, D), vector.shape

    nc = _get_nc()
    fp = _fingerprint(Wg, bg, Wf, bf, W1, b1, W2, b2, weight)
    if _NC_CACHE.get("shared_fp") != fp:
        _NC_CACHE["shared"] = _prep_shared(
            np.asarray(Wg, np.float32), bg, np.asarray(Wf, np.float32), bf,
            np.asarray(W1, np.float32), np.asarray(b1, np.float32),
            np.asarray(W2, np.float32), np.asarray(b2, np.float32), weight)
        _NC_CACHE["shared_fp"] = fp
    halves = _NC_CACHE["shared"]

    tokens = vector.reshape(B * S, D)
    xts = []
    for g in range(DP):
        xt = np.ascontiguousarray(tokens[g * T:(g + 1) * T].T)
        xts.append((xt, xt.astype(ml_dtypes.bfloat16)))
    in_maps = []
    for c in range(NCORES):
        h, g = divmod(c, DP)
        m = dict(halves[h])
        m["xf"], m["xb"] = xts[g]
        in_maps.append(m)

    trace = bool(os.environ.get("MOE_TRACE"))
    res = run_bass_kernel_spmd(nc, in_maps, core_ids=list(range(NCORES)),
                               trace=trace)
    if trace:
        LAST_EXEC_TIME_NS = res.exec_time_ns

    out0 = np.empty((B * S, H), np.float32)
    out1 = np.empty((B * S, H), np.float32)
    for g in range(DP):
        sl = slice(g * T, (g + 1) * T)
        out0[sl] = (res.results[g]["y0"].T.astype(np.float32)
                    + res.results[DP + g]["y0"].T.astype(np.float32))
        out1[sl] = (res.results[g]["y1"].T.astype(np.float32)
                    + res.results[DP + g]["y1"].T.astype(np.float32))
    return (np.ascontiguousarray(out0.reshape(B, S, H)),
            np.ascontiguousarray(out1.reshape(B, S, H)))
